# revision 1
# baseline (speedup 1.0000x reference)
"""Trainium2 Bass kernel for nn_Cheb_44693429682815.

ChebConv(K=1) stack == 3-layer MLP over 1M nodes (edge tensors unused):
    h = relu(x @ W0.T + b0); h = relu(h @ W1.T + b1); out = h @ W2.T  (b2 == 0)

Strategy (data-parallel over nodes, 8 cores):
  - Each core gets 125k rows, padded to 62 supertiles x 2048 rows.
  - Per supertile (2048 nodes -> 1024 free columns, 2 nodes stacked per
    partition-column): SWDGE DMA-in with inline fp32->bf16 cast, 8 PE
    transposes ([128,128] node-major -> feature-major, two 64-feature
    node groups stacked on 128 partitions), block-diagonal bf16 matmuls
    for layers 0/1 (N=512 each, fp32 PSUM), fused bias+relu on ACT/DVE,
    layer 2 as data-stationary matmuls that emit node-major output
    directly, ACT copy to SBUF, HWDGE DMA-out.
  - Layout is slot-remapped (node = r0 + 16*j + s) so both DMAs move
    >=1KB contiguous runs per partition.
"""

import numpy as np
import ml_dtypes

N_NODES = 1_000_000
C_IN, C_HID, C_OUT = 64, 64, 32
N_CORES = 8
ROWS_PER_CORE = N_NODES // N_CORES          # 125000
SUPER = 2048                                 # nodes per supertile
S_COLS = SUPER // 2                          # 1024 stacked free columns
SLOTS = SUPER // 128                         # 16 rows per partition
N_SUPER = (ROWS_PER_CORE + SUPER - 1) // SUPER   # 62
ROWS_PAD = N_SUPER * SUPER                   # 126976

_CACHE = {}


def _build_program(n_super):
    from contextlib import ExitStack

    import concourse.bass as bass  # noqa: F401
    import concourse.tile as tile
    import concourse.mybir as mybir
    from concourse import bacc

    f32 = mybir.dt.float32
    bf16 = mybir.dt.bfloat16
    rows = n_super * SUPER

    nc = bacc.Bacc(
        "TRN2", target_bir_lowering=False, debug=False, num_devices=N_CORES
    )
    x_d = nc.dram_tensor("x", [rows, C_IN], f32, kind="ExternalInput").ap()
    w0_d = nc.dram_tensor("bdw0t", [128, 128], bf16, kind="ExternalInput").ap()
    w1_d = nc.dram_tensor("bdw1t", [128, 128], bf16, kind="ExternalInput").ap()
    w2_d = nc.dram_tensor("bdw2t", [128, 64], bf16, kind="ExternalInput").ap()
    id_d = nc.dram_tensor("ident", [128, 128], bf16, kind="ExternalInput").ap()
    b0_d = nc.dram_tensor("b0s", [128, 1], f32, kind="ExternalInput").ap()
    b1_d = nc.dram_tensor("b1s", [128, 1], f32, kind="ExternalInput").ap()
    out_d = nc.dram_tensor("out", [rows, C_OUT], f32, kind="ExternalOutput").ap()

    relu = mybir.ActivationFunctionType.Relu
    add = mybir.AluOpType.add
    amax = mybir.AluOpType.max

    with tile.TileContext(nc) as tc:
        with ExitStack() as ctx:
            consts = ctx.enter_context(tc.tile_pool(name="consts", bufs=1))
            w0_sb = consts.tile([128, 128], bf16, tag="w0")
            w1_sb = consts.tile([128, 128], bf16, tag="w1")
            w2_sb = consts.tile([128, 64], bf16, tag="w2")
            id_sb = consts.tile([128, 128], bf16, tag="id")
            b0_sb = consts.tile([128, 1], f32, tag="b0")
            b1_sb = consts.tile([128, 1], f32, tag="b1")
            nc.sync.dma_start(w0_sb[:], w0_d)
            nc.sync.dma_start(w1_sb[:], w1_d)
            nc.sync.dma_start(w2_sb[:], w2_d)
            nc.sync.dma_start(id_sb[:], id_d)
            nc.sync.dma_start(b0_sb[:], b0_d)
            nc.sync.dma_start(b1_sb[:], b1_d)

            xt_pool = ctx.enter_context(tc.tile_pool(name="xt", bufs=4))
            xT_pool = ctx.enter_context(tc.tile_pool(name="xT", bufs=3))
            h_pool = ctx.enter_context(tc.tile_pool(name="h", bufs=3))
            osb_pool = ctx.enter_context(tc.tile_pool(name="osb", bufs=3))
            ph_pool = ctx.enter_context(
                tc.tile_pool(name="ph", bufs=2, space="PSUM")
            )
            px_pool = ctx.enter_context(
                tc.tile_pool(name="px", bufs=3, space="PSUM")
            )

            for st in range(n_super):
                r0 = st * SUPER
                # -------- load: x[r0 + 16j + s, c] -> xt[j, 64s + c] (bf16)
                xt = xt_pool.tile([128, S_COLS], bf16, tag="xt")
                nc.gpsimd.dma_start(
                    xt[:].rearrange("j (s c) -> j s c", s=SLOTS),
                    x_d[r0 : r0 + SUPER, :].rearrange(
                        "(j s) c -> j s c", j=128
                    ),
                )
                # -------- transpose: 8 chunks of [128,128] -> stacked x^T
                ps_xt = px_pool.tile([128, S_COLS], bf16, tag="px")
                for t in range(8):
                    sl = slice(128 * t, 128 * (t + 1))
                    nc.tensor.transpose(ps_xt[:, sl], xt[:, sl], id_sb[:])
                xT = xT_pool.tile([128, S_COLS], bf16, tag="xT")
                nc.vector.tensor_copy(xT[:], ps_xt[:])
                # -------- layer 0 (block-diag weights), bias+relu on ACT
                ps_h0 = ph_pool.tile([128, S_COLS], f32, tag="ph")
                nc.tensor.matmul(ps_h0[:, 0:512], w0_sb[:], xT[:, 0:512])
                nc.tensor.matmul(ps_h0[:, 512:1024], w0_sb[:], xT[:, 512:1024])
                h0 = h_pool.tile([128, S_COLS], bf16, tag="h0")
                nc.scalar.activation(h0[:], ps_h0[:], relu, bias=b0_sb[:])
                # -------- layer 1, bias+relu fused on DVE
                ps_h1 = ph_pool.tile([128, S_COLS], f32, tag="ph")
                nc.tensor.matmul(ps_h1[:, 0:512], w1_sb[:], h0[:, 0:512])
                nc.tensor.matmul(ps_h1[:, 512:1024], w1_sb[:], h0[:, 512:1024])
                h1 = h_pool.tile([128, S_COLS], bf16, tag="h1")
                nc.vector.tensor_scalar(h1[:], ps_h1[:], b1_sb[:], 0.0, add, amax)
                # -------- layer 2: data-stationary, node-major output
                ps_out = px_pool.tile([128, 512], f32, tag="px")
                for t in range(8):
                    nc.tensor.matmul(
                        ps_out[:, 64 * t : 64 * (t + 1)],
                        h1[:, 128 * t : 128 * (t + 1)],
                        w2_sb[:],
                    )
                osb = osb_pool.tile([128, 512], f32, tag="osb")
                nc.scalar.copy(osb[:], ps_out[:])
                # -------- store: osb[j, 32s + o] -> out[r0 + 16j + s, o]
                nc.sync.dma_start(
                    out_d[r0 : r0 + SUPER, :].rearrange(
                        "(j s) o -> j (s o)", j=128
                    ),
                    osb[:],
                )

    nc.compile()
    return nc


def get_program(n_super=N_SUPER):
    if n_super not in _CACHE:
        _CACHE[n_super] = _build_program(n_super)
    return _CACHE[n_super]


def make_const_inputs(W0, b0, W1, b1, W2):
    bf = ml_dtypes.bfloat16

    def bd(w):  # block_diag(w.T, w.T) as bf16
        wt = w.T.astype(np.float32)
        k, m = wt.shape
        out = np.zeros((2 * k, 2 * m), dtype=bf)
        out[:k, :m] = wt.astype(bf)
        out[k:, m:] = wt.astype(bf)
        return out

    return {
        "bdw0t": bd(W0),
        "bdw1t": bd(W1),
        "bdw2t": bd(W2),
        "ident": np.eye(128, dtype=bf),
        "b0s": np.concatenate([b0, b0]).astype(np.float32).reshape(128, 1),
        "b1s": np.concatenate([b1, b1]).astype(np.float32).reshape(128, 1),
    }


def kernel(x, edge_index, edge_weight, W0, b0, W1, b1, W2, b2, _trace=False):
    del edge_index, edge_weight, b2  # unused by ChebConv K=1 math
    from concourse.bass_utils import run_bass_kernel_spmd

    nc = get_program()
    consts = make_const_inputs(W0, b0, W1, b1, W2)
    x = np.asarray(x, dtype=np.float32)

    in_maps = []
    for i in range(N_CORES):
        shard = np.zeros((ROWS_PAD, C_IN), dtype=np.float32)
        shard[:ROWS_PER_CORE] = x[i * ROWS_PER_CORE : (i + 1) * ROWS_PER_CORE]
        in_maps.append({"x": shard, **consts})

    res = run_bass_kernel_spmd(
        nc, in_maps, core_ids=list(range(N_CORES)), trace=_trace
    )
    out = np.concatenate(
        [res.results[i]["out"][:ROWS_PER_CORE] for i in range(N_CORES)], axis=0
    )
    if _trace:
        kernel.last_results = res
    return out


# revision 2
# speedup vs baseline: 1.1521x; 1.1521x over previous
"""Trainium2 Bass kernel for nn_Cheb_44693429682815.

ChebConv(K=1) stack == 3-layer MLP over 1M nodes (edge tensors unused):
    h = relu(x @ W0.T + b0); h = relu(h @ W1.T + b1); out = h @ W2.T  (b2 == 0)

Strategy (data-parallel over nodes, 8 cores):
  - Host pre-casts x to bf16 and packs row pairs: xp[r, :] = [x[2r], x[2r+1]]
    (a [ROWS/2, 128] bf16 view of the same bytes).
  - Per 2048-node supertile: one HWDGE xbar DMA-transpose loads
    xT [128, 1024] = feature-major with node parity stacked on the
    partition halves. Layers 0/1 are block-diag(W.T, W.T) bf16 matmuls
    (N=512, fp32 PSUM) + fused bias+relu (ACT / DVE). Layer 2 is
    block-diag(W2.T) producing out^T stacked [64, 1024], copied to SBUF
    (split ACT/DVE) and stored contiguously into out_t [64, ROWS/2].
  - Host detangles: out[2j+g, o] = out_t[32 g + o, j].
  All DMAs move >=2KB contiguous runs per partition.
"""

import numpy as np
import ml_dtypes

N_NODES = 1_000_000
C_IN, C_HID, C_OUT = 64, 64, 32
N_CORES = 8
ROWS_PER_CORE = N_NODES // N_CORES          # 125000
SUPER = 2048                                 # nodes per supertile
S_COLS = SUPER // 2                          # 1024 stacked free columns
N_SUPER = (ROWS_PER_CORE + SUPER - 1) // SUPER   # 62
ROWS_PAD = N_SUPER * SUPER                   # 126976

_CACHE = {}


def _build_program(n_super):
    from contextlib import ExitStack

    import concourse.bass as bass  # noqa: F401
    import concourse.tile as tile
    import concourse.mybir as mybir
    from concourse import bacc

    f32 = mybir.dt.float32
    bf16 = mybir.dt.bfloat16
    half_rows = n_super * S_COLS

    nc = bacc.Bacc(
        "TRN2", target_bir_lowering=False, debug=False, num_devices=N_CORES
    )
    xp_d = nc.dram_tensor("xp", [half_rows, 128], bf16, kind="ExternalInput").ap()
    w0_d = nc.dram_tensor("bdw0t", [128, 128], bf16, kind="ExternalInput").ap()
    w1_d = nc.dram_tensor("bdw1t", [128, 128], bf16, kind="ExternalInput").ap()
    w2_d = nc.dram_tensor("bdw2t", [128, 64], bf16, kind="ExternalInput").ap()
    b0_d = nc.dram_tensor("b0s", [128, 1], f32, kind="ExternalInput").ap()
    b1_d = nc.dram_tensor("b1s", [128, 1], f32, kind="ExternalInput").ap()
    ot_d = nc.dram_tensor("out_t", [64, half_rows], f32, kind="ExternalOutput").ap()

    relu = mybir.ActivationFunctionType.Relu
    add = mybir.AluOpType.add
    amax = mybir.AluOpType.max

    with tile.TileContext(nc) as tc:
        with ExitStack() as ctx:
            consts = ctx.enter_context(tc.tile_pool(name="consts", bufs=1))
            w0_sb = consts.tile([128, 128], bf16, tag="w0")
            w1_sb = consts.tile([128, 128], bf16, tag="w1")
            w2_sb = consts.tile([128, 64], bf16, tag="w2")
            b0_sb = consts.tile([128, 1], f32, tag="b0")
            b1_sb = consts.tile([128, 1], f32, tag="b1")
            nc.sync.dma_start(w0_sb[:], w0_d)
            nc.sync.dma_start(w1_sb[:], w1_d)
            nc.sync.dma_start(w2_sb[:], w2_d)
            nc.sync.dma_start(b0_sb[:], b0_d)
            nc.sync.dma_start(b1_sb[:], b1_d)

            xT_pool = ctx.enter_context(tc.tile_pool(name="xT", bufs=4))
            h0_pool = ctx.enter_context(tc.tile_pool(name="h0", bufs=3))
            h1_pool = ctx.enter_context(tc.tile_pool(name="h1", bufs=3))
            osb_pool = ctx.enter_context(tc.tile_pool(name="osb", bufs=3))
            ps_pool = ctx.enter_context(
                tc.tile_pool(name="ps", bufs=3, space="PSUM")
            )

            for st in range(n_super):
                c0 = st * S_COLS
                # ---- load + transpose: xT[p, j] = xp[c0+j, p] (bf16)
                xT = xT_pool.tile([128, S_COLS], bf16, tag="xT")
                nc.sync.dma_start(
                    xT[:], xp_d[c0 : c0 + S_COLS, :], transpose=True
                )
                # ---- layer 0: block-diag weights, bias+relu on ACT
                ps_h0 = ps_pool.tile([128, S_COLS], f32, tag="ps")
                nc.tensor.matmul(ps_h0[:, 0:512], w0_sb[:], xT[:, 0:512])
                nc.tensor.matmul(ps_h0[:, 512:1024], w0_sb[:], xT[:, 512:1024])
                h0 = h0_pool.tile([128, S_COLS], bf16, tag="h0")
                nc.scalar.activation(h0[:], ps_h0[:], relu, bias=b0_sb[:])
                # ---- layer 1: bias+relu fused on DVE
                ps_h1 = ps_pool.tile([128, S_COLS], f32, tag="ps")
                nc.tensor.matmul(ps_h1[:, 0:512], w1_sb[:], h0[:, 0:512])
                nc.tensor.matmul(ps_h1[:, 512:1024], w1_sb[:], h0[:, 512:1024])
                h1 = h1_pool.tile([128, S_COLS], bf16, tag="h1")
                nc.vector.tensor_scalar(h1[:], ps_h1[:], b1_sb[:], 0.0, add, amax)
                # ---- layer 2: out^T stacked [64, 1024]
                ps_o = ps_pool.tile([64, S_COLS], f32, tag="ps")
                nc.tensor.matmul(ps_o[:, 0:512], w2_sb[:], h1[:, 0:512])
                nc.tensor.matmul(ps_o[:, 512:1024], w2_sb[:], h1[:, 512:1024])
                osb = osb_pool.tile([64, S_COLS], f32, tag="osb")
                nc.scalar.copy(osb[:, 0:512], ps_o[:, 0:512])
                nc.vector.tensor_copy(osb[:, 512:1024], ps_o[:, 512:1024])
                # ---- store: out_t[32g + o, c0 + j] = out[r0 + 2j + g, o]
                nc.sync.dma_start(ot_d[:, c0 : c0 + S_COLS], osb[:])

    nc.compile()
    return nc


def get_program(n_super=N_SUPER):
    if n_super not in _CACHE:
        _CACHE[n_super] = _build_program(n_super)
    return _CACHE[n_super]


def make_const_inputs(W0, b0, W1, b1, W2):
    bf = ml_dtypes.bfloat16

    def bd(w):  # block_diag(w.T, w.T) as bf16
        wt = np.asarray(w, dtype=np.float32).T
        k, m = wt.shape
        out = np.zeros((2 * k, 2 * m), dtype=bf)
        out[:k, :m] = wt.astype(bf)
        out[k:, m:] = wt.astype(bf)
        return out

    b0 = np.asarray(b0, np.float32)
    b1 = np.asarray(b1, np.float32)
    return {
        "bdw0t": bd(W0),
        "bdw1t": bd(W1),
        "bdw2t": bd(W2),
        "b0s": np.concatenate([b0, b0]).reshape(128, 1).copy(),
        "b1s": np.concatenate([b1, b1]).reshape(128, 1).copy(),
    }


def make_shards(x):
    """Per-core packed bf16 input: xp[r] = [x[2r], x[2r+1]] (padded)."""
    bf = ml_dtypes.bfloat16
    x = np.asarray(x, dtype=np.float32)
    shards = []
    for i in range(N_CORES):
        xs = np.zeros((ROWS_PAD, C_IN), dtype=bf)
        xs[:ROWS_PER_CORE] = x[i * ROWS_PER_CORE : (i + 1) * ROWS_PER_CORE]
        shards.append(xs.reshape(ROWS_PAD // 2, 128))
    return shards


def gather_output(results):
    """out[2j+g, o] = out_t[32g + o, j] per core; concat and trim."""
    outs = []
    for i in range(N_CORES):
        ot = results[i]["out_t"]
        oc = np.empty((ROWS_PAD, C_OUT), dtype=np.float32)
        oc[0::2] = ot[:32].T
        oc[1::2] = ot[32:].T
        outs.append(oc[:ROWS_PER_CORE])
    return np.concatenate(outs, axis=0)


def kernel(x, edge_index, edge_weight, W0, b0, W1, b1, W2, b2, _trace=False):
    del edge_index, edge_weight, b2  # unused by ChebConv K=1 math
    from concourse.bass_utils import run_bass_kernel_spmd

    nc = get_program()
    consts = make_const_inputs(W0, b0, W1, b1, W2)
    shards = make_shards(x)
    in_maps = [{"xp": shards[i], **consts} for i in range(N_CORES)]

    res = run_bass_kernel_spmd(
        nc, in_maps, core_ids=list(range(N_CORES)), trace=_trace
    )
    if _trace:
        kernel.last_results = res
    return gather_output(res.results)


# revision 5
# speedup vs baseline: 1.4072x; 1.2215x over previous
"""Trainium2 Bass kernel for nn_Cheb_44693429682815.

ChebConv(K=1) stack == 3-layer MLP over 1M nodes (edge tensors unused):
    h = relu(x @ W0.T + b0); h = relu(h @ W1.T + b1); out = h @ W2.T  (b2 == 0)

Strategy (data-parallel over nodes, 8 cores):
  - Host pre-casts x to bf16 and packs row pairs: xp[r, :] = [x[2r], x[2r+1]]
    (a [ROWS/2, 128] bf16 view of the same bytes).
  - Per 2048-node supertile: one HWDGE xbar DMA-transpose loads
    xT [128, 1024] = feature-major with node parity stacked on the
    partition halves. Layers 0/1 are block-diag(W.T, W.T) bf16 matmuls
    (N=512, fp32 PSUM) + fused bias+relu (ACT / DVE). Layer 2 is
    block-diag(W2.T) producing out^T stacked [64, 1024], copied to SBUF
    (split ACT/DVE) and stored contiguously into out_t [64, ROWS/2].
  - Host detangles: out[2j+g, o] = out_t[32 g + o, j].
  All DMAs move >=2KB contiguous runs per partition.
"""

import numpy as np
import ml_dtypes

N_NODES = 1_000_000
C_IN, C_HID, C_OUT = 64, 64, 32
N_CORES = 8
ROWS_PER_CORE = N_NODES // N_CORES          # 125000
SUPER = 2048                                 # nodes per supertile
S_COLS = SUPER // 2                          # 1024 stacked free columns
N_SUPER = (ROWS_PER_CORE + SUPER - 1) // SUPER   # 62
ROWS_PAD = N_SUPER * SUPER                   # 126976

_CACHE = {}


def _build_program(n_super):
    from contextlib import ExitStack

    import concourse.bass as bass  # noqa: F401
    import concourse.tile as tile
    import concourse.mybir as mybir
    from concourse import bacc

    f32 = mybir.dt.float32
    bf16 = mybir.dt.bfloat16
    half_rows = n_super * S_COLS

    nc = bacc.Bacc(
        "TRN2", target_bir_lowering=False, debug=False, num_devices=N_CORES
    )
    xp_d = nc.dram_tensor("xp", [half_rows, 128], bf16, kind="ExternalInput").ap()
    w0_d = nc.dram_tensor("bdw0t", [128, 128], bf16, kind="ExternalInput").ap()
    w1_d = nc.dram_tensor("bdw1t", [128, 128], bf16, kind="ExternalInput").ap()
    w2_d = nc.dram_tensor("bdw2t", [128, 64], bf16, kind="ExternalInput").ap()
    b0_d = nc.dram_tensor("b0s", [128, 1], f32, kind="ExternalInput").ap()
    b1_d = nc.dram_tensor("b1s", [128, 1], f32, kind="ExternalInput").ap()
    ot_d = nc.dram_tensor(
        "out_t", [128, n_super * 512], f32, kind="ExternalOutput"
    ).ap()

    relu = mybir.ActivationFunctionType.Relu
    add = mybir.AluOpType.add
    amax = mybir.AluOpType.max

    with tile.TileContext(nc) as tc:
        with ExitStack() as ctx:
            consts = ctx.enter_context(tc.tile_pool(name="consts", bufs=1))
            w0_sb = consts.tile([128, 128], bf16, tag="w0")
            w1_sb = consts.tile([128, 128], bf16, tag="w1")
            w2_sb = consts.tile([128, 64], bf16, tag="w2")
            b0_sb = consts.tile([128, 1], f32, tag="b0")
            b1_sb = consts.tile([128, 1], f32, tag="b1")
            nc.sync.dma_start(w0_sb[:], w0_d)
            nc.sync.dma_start(w1_sb[:], w1_d)
            nc.sync.dma_start(w2_sb[:], w2_d)
            nc.sync.dma_start(b0_sb[:], b0_d)
            nc.sync.dma_start(b1_sb[:], b1_d)

            xT_pool = ctx.enter_context(tc.tile_pool(name="xT", bufs=4))
            h0_pool = ctx.enter_context(tc.tile_pool(name="h0", bufs=6))
            h1_pool = ctx.enter_context(tc.tile_pool(name="h1", bufs=6))
            osb_pool = ctx.enter_context(tc.tile_pool(name="osb", bufs=3))
            ph0_pool = ctx.enter_context(
                tc.tile_pool(name="ph0", bufs=3, space="PSUM")
            )
            ph1_pool = ctx.enter_context(
                tc.tile_pool(name="ph1", bufs=3, space="PSUM")
            )
            po_pool = ctx.enter_context(
                tc.tile_pool(name="po", bufs=2, space="PSUM")
            )

            for st in range(n_super):
                c0 = st * S_COLS
                # ---- load + transpose: xT[p, j] = xp[c0+j, p] (bf16)
                xT = xT_pool.tile([128, S_COLS], bf16, tag="xT")
                nc.sync.dma_start(
                    xT[:], xp_d[c0 : c0 + S_COLS, :], transpose=True
                )
                # L2 outputs of the two 512-col units pack into one
                # [128, 512] PSUM tile via the col-group offset (u*64).
                po = po_pool.tile([128, 512], f32, tag="po")
                osb = osb_pool.tile([128, 512], f32, tag="osb")
                for u in range(2):
                    cs = slice(512 * u, 512 * (u + 1))
                    ps_h0 = ph0_pool.tile([128, 512], f32, tag="ph0")
                    nc.tensor.matmul(ps_h0[:], w0_sb[:], xT[:, cs])
                    h0 = h0_pool.tile([128, 512], bf16, tag="h0")
                    nc.scalar.activation(h0[:], ps_h0[:], relu, bias=b0_sb[:])
                    ps_h1 = ph1_pool.tile([128, 512], f32, tag="ph1")
                    nc.tensor.matmul(ps_h1[:], w1_sb[:], h0[:])
                    h1 = h1_pool.tile([128, 512], bf16, tag="h1")
                    nc.vector.tensor_scalar(
                        h1[:], ps_h1[:], b1_sb[:], 0.0, add, amax
                    )
                    nc.tensor.matmul(po[64 * u : 64 * (u + 1), :], w2_sb[:], h1[:])
                # ---- evacuate + store:
                # osb[64u + 32g + o, jj] = out[r0 + 1024u + 2jj + g, o]
                nc.scalar.copy(osb[:], po[:])
                nc.sync.dma_start(ot_d[:, st * 512 : (st + 1) * 512], osb[:])

    nc.compile()
    return nc


def get_program(n_super=N_SUPER):
    if n_super not in _CACHE:
        _CACHE[n_super] = _build_program(n_super)
    return _CACHE[n_super]


def make_const_inputs(W0, b0, W1, b1, W2):
    bf = ml_dtypes.bfloat16

    def bd(w):  # block_diag(w.T, w.T) as bf16
        wt = np.asarray(w, dtype=np.float32).T
        k, m = wt.shape
        out = np.zeros((2 * k, 2 * m), dtype=bf)
        out[:k, :m] = wt.astype(bf)
        out[k:, m:] = wt.astype(bf)
        return out

    b0 = np.asarray(b0, np.float32)
    b1 = np.asarray(b1, np.float32)
    return {
        "bdw0t": bd(W0),
        "bdw1t": bd(W1),
        "bdw2t": bd(W2),
        "b0s": np.concatenate([b0, b0]).reshape(128, 1).copy(),
        "b1s": np.concatenate([b1, b1]).reshape(128, 1).copy(),
    }


def make_shards(x):
    """Per-core packed bf16 input: xp[r] = [x[2r], x[2r+1]] (padded)."""
    bf = ml_dtypes.bfloat16
    x = np.asarray(x, dtype=np.float32)
    shards = []
    for i in range(N_CORES):
        xs = np.zeros((ROWS_PAD, C_IN), dtype=bf)
        xs[:ROWS_PER_CORE] = x[i * ROWS_PER_CORE : (i + 1) * ROWS_PER_CORE]
        shards.append(xs.reshape(ROWS_PAD // 2, 128))
    return shards


def gather_output(results):
    """node st*2048 + 1024u + 2jj + g, feature o <- out_t[64u+32g+o, st*512+jj]."""
    outs = []
    for i in range(N_CORES):
        ot = np.asarray(results[i]["out_t"])
        n_super = ot.shape[1] // 512
        ot5 = ot.reshape(2, 2, 32, n_super, 512)
        oc = np.ascontiguousarray(
            np.transpose(ot5, (3, 0, 4, 1, 2)).reshape(n_super * 2048, C_OUT)
        )
        outs.append(oc[:ROWS_PER_CORE])
    return np.concatenate(outs, axis=0)


def kernel(x, edge_index, edge_weight, W0, b0, W1, b1, W2, b2, _trace=False):
    del edge_index, edge_weight, b2  # unused by ChebConv K=1 math
    from concourse.bass_utils import run_bass_kernel_spmd

    nc = get_program()
    consts = make_const_inputs(W0, b0, W1, b1, W2)
    shards = make_shards(x)
    in_maps = [{"xp": shards[i], **consts} for i in range(N_CORES)]

    res = run_bass_kernel_spmd(
        nc, in_maps, core_ids=list(range(N_CORES)), trace=_trace
    )
    if _trace:
        kernel.last_results = res
    return gather_output(res.results)
